# revision 10
# baseline (speedup 1.0000x reference)
"""Trainium2 Bass kernel for CosineSimilarityWeightedAverage.

reference:
  input [B=4, C=4096, D=64] f32
  in_n = input / ||input||_row
  cos  = in_n @ in_n.T per batch            [B, C, C]
  attn = softmax(cos / 0.1, axis=-1)
  out  = (attn @ weight) * weight_global * input + (attn @ bias) * bias_global

Sharding: 8 cores = (batch b = core//2) x (query half h = core%2, 2048 rows).
Each core sees all 4096 keys of its batch (permuted so its own queries come
first) and computes 2048 output rows.

Per-core dataflow (v2):
  - host supplies layout-only transforms: f16 cast of x, f16 transposed keys
    xkT [64, 4096] (so no on-device key transposes), and [W|bias] pre-packed
    in fp8e5 DoubleRow layout [128, 16, 2, 128].
  - keys stay UNNORMALIZED; the 10/||k|| factor (temperature folded) is a
    per-partition scalar applied inside the exp (activation scale AP / DVE
    tensor_scalar scalar AP) in the [k, q] score layout.
  - queries are normalized on device (16 tiles) and PE-transposed.
  - stage 1: st[k, q] = xkT.T @ qnT per k-tile (f16, K=64).
  - exp split across engines: ACT runs native Exp -> fp8e5; DVE fabricates
    the e5m2 bits with the exp2 bit trick (i8 = 4*log2e*scale*st + 60.5,
    truncated, bitcast to fp8e5). Both cancel exactly in softmax.
  - stage 2 + denominator: fp8e5 DoubleRow matmuls (2 k-tiles per matmul,
    0.5 cycles/row): attn-num [128cols, 512q] and den [1, 512q] accumulate
    in PSUM f32 over the 16 k-tile pairs.
  - finalize per 512-query chunk: reciprocal, partition-broadcast, normalize,
    PE transpose back to [q, d], out = avgW*(wg*x) + avgB*bg, DMA out.
"""

import numpy as np

B = 4
C = 4096
D = 64
NCORES = 8
CQ = C // 2          # queries per core
KT = C // 128        # 32 k-tiles
QT = CQ // 128       # 16 q-tiles
NJ = KT // 2         # 16 k-tile pairs
LOG2E = 1.4426950408889634

# exp routing: k-tile pairs handled by the DVE bit-trick (rest go to ACT).
# Strict alternation so consecutive pairs overlap on different engines.
DVE_PAIRS = frozenset({1, 3, 5, 7, 9, 11, 13, 15})

_CACHE = {}


def _build():
    import concourse.bass as bass
    import concourse.bacc as bacc
    import concourse.mybir as mybir
    import concourse.tile as tile
    from concourse.masks import make_identity

    f32 = mybir.dt.float32
    f16 = mybir.dt.float16
    f8 = mybir.dt.float8e5
    i8 = mybir.dt.int8
    AF = mybir.ActivationFunctionType
    DR = mybir.MatmulPerfMode.DoubleRow
    ALU = mybir.AluOpType

    nc = bacc.Bacc(None, target_bir_lowering=False)
    xq16 = nc.dram_tensor("xq16", [CQ, D], f16, kind="ExternalInput")
    xk16 = nc.dram_tensor("xk16", [C, D], f16, kind="ExternalInput")
    xkT = nc.dram_tensor("xkT", [D, C], f16, kind="ExternalInput")
    wsb8 = nc.dram_tensor("wsb8", [128, NJ, 2, 2 * D], f8, kind="ExternalInput")
    wg = nc.dram_tensor("wg", [CQ, D], f32, kind="ExternalInput")
    bg = nc.dram_tensor("bg", [CQ, D], f32, kind="ExternalInput")
    out = nc.dram_tensor("out", [CQ, D], f32, kind="ExternalOutput")

    with tile.TileContext(nc) as tc:
        with (
            tc.tile_pool(name="singles", bufs=1) as singles,
            tc.tile_pool(name="sb", bufs=2) as sb,
            tc.tile_pool(name="exp", bufs=6) as expp,
            tc.tile_pool(name="fin", bufs=2) as fin,
            tc.tile_pool(name="stage", bufs=4, space="PSUM") as stage,
            tc.tile_pool(name="otp", bufs=1, space="PSUM") as otp,
            tc.tile_pool(name="acc", bufs=2, space="PSUM") as accp,
            tc.tile_pool(name="den", bufs=1, space="PSUM") as denp,
        ):
            # ---------------- identity (no DMA deps — emit first) ----------
            identity = singles.tile([128, 128], f32)
            make_identity(nc, identity)
            identity16 = singles.tile([128, 128], f16)
            nc.vector.tensor_copy(out=identity16, in_=identity)
            ones8 = singles.tile([128, 2, 32], f8)
            nc.vector.memset(ones8, 1.0)

            # ---------------- loads ----------------
            # sync/HWDGE queue, critical path first: xq16 (q norms +
            # transposes), then xk16/xkT interleaved (k norms feed the exp
            # scales, xkT is the stage-1 lhsT). Params go on the Pool SWDGE
            # queue so they stream in parallel (Pool is idle mid-init).
            xq_s = singles.tile([128, QT, D], f16)
            nc.sync.dma_start(out=xq_s, in_=xq16.rearrange("(t p) d -> p t d", p=128))
            xk_s = singles.tile([128, KT, D], f16)
            xk_r = xk16.rearrange("(t p) d -> p t d", p=128)
            xkT_s = singles.tile([64, KT, 128], f16)
            xkT_r = xkT.rearrange("d (t k) -> d t k", k=128)
            cs4 = [slice(8 * c, 8 * (c + 1)) for c in range(4)]
            nc.sync.dma_start(out=xk_s[:, cs4[0], :], in_=xk_r[:, cs4[0], :])
            nc.sync.dma_start(out=xkT_s[:, cs4[0], :], in_=xkT_r[:, cs4[0], :])
            nc.sync.dma_start(out=xkT_s[:, cs4[1], :], in_=xkT_r[:, cs4[1], :])
            nc.sync.dma_start(out=xk_s[:, cs4[1], :], in_=xk_r[:, cs4[1], :])
            nc.sync.dma_start(out=xkT_s[:, cs4[2], :], in_=xkT_r[:, cs4[2], :])
            nc.sync.dma_start(out=xk_s[:, cs4[2], :], in_=xk_r[:, cs4[2], :])
            nc.sync.dma_start(out=xkT_s[:, cs4[3], :], in_=xkT_r[:, cs4[3], :])
            nc.sync.dma_start(out=xk_s[:, cs4[3], :], in_=xk_r[:, cs4[3], :])
            wsb_s = singles.tile([128, NJ, 2, 2 * D], f8)
            nc.gpsimd.dma_start(out=wsb_s, in_=wsb8[:, :, :, :])
            wgs = singles.tile([128, QT, D], f32)
            nc.gpsimd.dma_start(out=wgs, in_=wg.rearrange("(t p) d -> p t d", p=128))
            bgs = singles.tile([128, QT, D], f32)
            nc.gpsimd.dma_start(out=bgs, in_=bg.rearrange("(t p) d -> p t d", p=128))

            # ---------------- q norms + transposes (critical path) --------
            qsq = singles.tile([128, QT], f32)
            for c in range(2):
                cs = slice(8 * c, 8 * (c + 1))
                qtmp = sb.tile([128, 8, D], f32, tag="sqt", name=f"qtmp{c}")
                nc.vector.tensor_mul(qtmp, xq_s[:, cs, :], xq_s[:, cs, :])
                nc.vector.reduce_sum(out=qsq[:, cs], in_=qtmp, axis=mybir.AxisListType.X)
            qscale = singles.tile([128, QT], f32)
            nc.scalar.activation(out=qscale, in_=qsq, func=AF.Sqrt, scale=1.0)
            nc.vector.reciprocal(out=qscale, in_=qscale)

            # normalized queries (f16) + PE transpose to [64, q], pipelined
            # per tile; bank copies split ACT/DVE.
            qn16 = singles.tile([128, QT, D], f16)
            qnT = singles.tile([64, QT, 128], f16)
            for bk in range(2):
                pt = stage.tile([64, 8, 128], f16, tag="stage", name=f"ptq{bk}")
                for s in range(8):
                    t = 8 * bk + s
                    nc.vector.tensor_scalar_mul(
                        out=qn16[:, t, :], in0=xq_s[:, t, :],
                        scalar1=qscale[:, t : t + 1],
                    )
                    nc.tensor.transpose(pt[:, s, :], qn16[:, t, :], identity16)
                if bk == 0:
                    nc.scalar.copy(out=qnT[:, 0:8, :], in_=pt)
                else:
                    nc.vector.tensor_copy(out=qnT[:, 8:16, :], in_=pt)

            # ---------------- k norms ----------------
            # kinv10 = 10/||k|| (temperature folded) for the ACT exp scale,
            # kdve = kinv10 * 4*log2e for the DVE exp bit-trick.
            ksq = singles.tile([128, KT], f32)
            kinv10 = singles.tile([128, KT], f32)
            kdve = singles.tile([128, KT], f32)
            for c in range(4):
                cs = cs4[c]
                ktmp = sb.tile([128, 8, D], f32, tag="sqt", name=f"ktmp{c}")
                nc.vector.tensor_mul(ktmp, xk_s[:, cs, :], xk_s[:, cs, :])
                nc.vector.reduce_sum(out=ksq[:, cs], in_=ktmp, axis=mybir.AxisListType.X)
                # sqrt(0.01*s) = ||k||/10 ; reciprocal -> 10/||k||
                nc.scalar.activation(
                    out=kinv10[:, cs], in_=ksq[:, cs], func=AF.Sqrt, scale=0.01
                )
                nc.vector.reciprocal(out=kinv10[:, cs], in_=kinv10[:, cs])
                nc.vector.tensor_scalar_mul(
                    out=kdve[:, cs], in0=kinv10[:, cs], scalar1=4.0 * LOG2E
                )

            # winp = wg * x on Pool (keeps the DVE queue free; wgs arrives
            # on the Pool DMA queue well before the first finalize)
            winp = singles.tile([128, QT, D], f32)
            nc.gpsimd.tensor_mul(winp, wgs, xq_s)

            # ---------------- main loop ----------------
            # Quarter-sweeps: one 512-query chunk at a time. PSUM budget
            # (8 banks): stage 4x[128,512]=4, ot 1, acc 2x[128,512]=2,
            # den 1. Software pipeline with 1-iteration skew; half-pair
            # st tiles keep 2 pairs in flight so the alternating ACT/DVE
            # exps overlap.
            for qc in range(4):
                acc_ps = accp.tile([128, 512], f32, tag="acc", name=f"acc{qc}")
                den_ps = denp.tile([32, 512], f32, tag="den", name=f"den{qc}")
                rhs = qnT[:, 4 * qc : 4 * qc + 4, :]

                exps = {}
                for j in range(NJ + 1):
                    if j < NJ:
                        e8 = expp.tile([128, 2, 512], f8, tag="exp",
                                       name=f"e{qc}_{j}")
                        e8i = e8.bitcast(i8)
                        for par in range(2):
                            kt = 2 * j + par
                            st = stage.tile([128, 512], f32, tag="stage",
                                            name=f"st{qc}_{kt}")
                            nc.tensor.matmul(
                                st, lhsT=xkT_s[:, kt, :], rhs=rhs,
                                start=True, stop=True,
                            )
                            if j in DVE_PAIRS:
                                nc.vector.tensor_scalar(
                                    out=e8i[:, par, :], in0=st,
                                    scalar1=kdve[:, kt : kt + 1], scalar2=60.5,
                                    op0=ALU.mult, op1=ALU.add,
                                )
                            else:
                                nc.scalar.activation(
                                    out=e8[:, par, :], in_=st,
                                    func=AF.Exp, scale=kinv10[:, kt : kt + 1],
                                )
                        exps[j] = e8
                    if j > 0:
                        jj = j - 1
                        e = exps.pop(jj)
                        nc.tensor.matmul(
                            acc_ps, lhsT=wsb_s[:, jj], rhs=e, perf_mode=DR,
                            start=(jj == 0), stop=(jj == NJ - 1),
                            skip_group_check=True,
                        )
                        nc.tensor.matmul(
                            den_ps, lhsT=ones8, rhs=e, perf_mode=DR,
                            start=(jj == 0), stop=(jj == NJ - 1),
                            skip_group_check=True,
                        )

                # ---------------- finalize ----------------
                # reciprocal on DVE (fires right after the last exp, freeing
                # den for the next chunk); broadcast + elementwise on Pool so
                # DVE/ACT stay on exp work.
                rinv = fin.tile([1, 512], f32, tag="rinv")
                nc.vector.reciprocal(out=rinv, in_=den_ps[0:1, :])
                rb = fin.tile([128, 512], f32, tag="rb")
                nc.gpsimd.partition_broadcast(rb, rinv)
                accs = fin.tile([128, 512], f32, tag="accs")
                nc.vector.tensor_mul(accs, acc_ps, rb)
                ot = otp.tile([128, 512], f32, tag="ot", name=f"ot{qc}")
                for s in range(4):
                    nc.tensor.transpose(
                        ot[:, s * 128 : (s + 1) * 128],
                        accs[:, s * 128 : (s + 1) * 128],
                        identity,
                    )
                ots = fin.tile([128, 512], f32, tag="ots")
                nc.scalar.copy(out=ots, in_=ot)
                ot4 = ots.rearrange("p (s k) -> p s k", s=4)
                qs = slice(4 * qc, 4 * qc + 4)
                t1 = fin.tile([128, 4, D], f32, tag="t1")
                nc.gpsimd.tensor_mul(t1, ot4[:, :, 0:64], winp[:, qs, :])
                t2 = fin.tile([128, 4, D], f32, tag="t2")
                nc.gpsimd.tensor_mul(t2, ot4[:, :, 64:128], bgs[:, qs, :])
                onat = fin.tile([128, 4, D], f32, tag="onat")
                nc.gpsimd.tensor_add(onat, t1, t2)
                nc.sync.dma_start(
                    out=out.rearrange("(t p) d -> p t d", p=128)[:, qs, :],
                    in_=onat,
                )

    nc.compile()
    return nc


def _get_nc():
    if "nc" not in _CACHE:
        _CACHE["nc"] = _build()
    return _CACHE["nc"]


def _make_in_maps(input, weight, bias, weight_global, bias_global):
    import ml_dtypes

    f8 = ml_dtypes.float8_e5m2
    input = np.ascontiguousarray(np.asarray(input, dtype=np.float32))
    ones = lambda: np.ones((C, D), np.float32)
    weight = ones() if weight is None else np.asarray(weight, np.float32)
    bias = np.zeros((C, D), np.float32) if bias is None else np.asarray(bias, np.float32)
    weight_global = ones() if weight_global is None else np.asarray(weight_global, np.float32)
    bias_global = ones() if bias_global is None else np.asarray(bias_global, np.float32)
    wcat = np.concatenate([weight, bias], axis=1)  # [C, 128]

    # per-half key permutation (own queries first) + DoubleRow fp8 layout:
    # wsb8[p, j, h, c] = wcat_perm[(2j+h)*128 + p, c]
    def dr_pack(wc):
        return np.ascontiguousarray(
            wc.reshape(NJ, 2, 128, 2 * D).transpose(2, 0, 1, 3).astype(f8)
        )

    wsb8_h = [
        dr_pack(wcat),
        dr_pack(np.concatenate([wcat[CQ:], wcat[:CQ]], axis=0)),
    ]

    in_maps = []
    for core in range(NCORES):
        b, h = divmod(core, 2)
        sl = slice(h * CQ, (h + 1) * CQ)
        xb = input[b]
        xperm = xb if h == 0 else np.concatenate([xb[CQ:], xb[:CQ]], axis=0)
        xk16 = np.ascontiguousarray(xperm.astype(np.float16))
        in_maps.append({
            "xq16": xk16[:CQ].copy(),
            "xk16": xk16,
            "xkT": np.ascontiguousarray(xk16.T),
            "wsb8": wsb8_h[h],
            "wg": np.ascontiguousarray(weight_global[sl]),
            "bg": np.ascontiguousarray(bias_global[sl]),
        })
    return in_maps


def _run(in_maps, **kw):
    from concourse.bass_utils import run_bass_kernel_spmd
    nc = _get_nc()
    return run_bass_kernel_spmd(nc, in_maps, core_ids=list(range(NCORES)), **kw)


def kernel(input, weight=None, bias=None, weight_global=None, bias_global=None,
           **_ignored):
    in_maps = _make_in_maps(input, weight, bias, weight_global, bias_global)
    res = _run(in_maps)
    out = np.empty((B, C, D), np.float32)
    for core in range(NCORES):
        b, h = divmod(core, 2)
        out[b, h * CQ : (h + 1) * CQ] = res.results[core]["out"]
    return out
